# revision 1
# baseline (speedup 1.0000x reference)
"""Causal linear attention (fast_transformers style) on 8 Trainium2 cores.

query (8, 512, 64, 128) f32. Data-parallel: one batch element per core.
Per (batch, node) sequence of L=512 tokens: project q/k/v with 128x128
weights, phi(x)=elu(x)+1, causal linear attention via chunked scan
(C=128 intra-chunk masked matmul + inter-chunk running KV state).

Wire strategy: the axon tunnel moves ~75 MB/s serialized and is the whole
cost, so ship int8 with per-token scales both directions (34MB up, 34MB
down; measured end-to-end rel l2 err ~1e-2 vs the 2e-2 gate). Matmuls run
fp16 with fp32 PSUM accumulation; constants and the output zero-buffers
stay resident on device across calls.
"""

import numpy as np

HEADS = 8
E = 16
EPS = 1e-6
L = 512
NSEQ = 64
F = 128
CH = HEADS * E  # 128 output channels
C = 128         # time chunk
NC = L // C
W17 = 17 * HEADS  # 136: per-head [num(16) | den(1)] interleaved width
AMAX_FLOOR = 1e-4


def build_nc(n_seq=NSEQ, debug=False):
    """Build the per-core Bass module. Parametrized n_seq for simulation."""
    from contextlib import ExitStack

    import concourse.bacc as bacc
    import concourse.mybir as mybir
    import concourse.tile as tile

    i8 = mybir.dt.int8
    f16 = mybir.dt.float16
    f32 = mybir.dt.float32
    Relu = mybir.ActivationFunctionType.Relu
    Exp = mybir.ActivationFunctionType.Exp
    AluOp = mybir.AluOpType
    AX = mybir.AxisListType.X

    ncol = n_seq * NC  # scale columns: one per (n, c) tile

    nc = bacc.Bacc(
        "TRN2",
        target_bir_lowering=False,
        debug=debug,
        enable_asserts=False,
        num_devices=8,
    )

    xq = nc.dram_tensor("xq", (L * n_seq, F), i8, kind="ExternalInput").ap()
    xsc = nc.dram_tensor("xsc", (C, ncol), f32, kind="ExternalInput").ap()
    wq = nc.dram_tensor("wq", (F, CH), f16, kind="ExternalInput").ap()
    wk = nc.dram_tensor("wk", (F, CH), f16, kind="ExternalInput").ap()
    wv = nc.dram_tensor("wv", (F, CH), f16, kind="ExternalInput").ap()
    bq = nc.dram_tensor("bq", (CH,), f16, kind="ExternalInput").ap()
    bk = nc.dram_tensor("bk", (CH,), f16, kind="ExternalInput").ap()
    bv = nc.dram_tensor("bv", (CH,), f16, kind="ExternalInput").ap()
    cmask = nc.dram_tensor("cmask", (C, C), f16, kind="ExternalInput").ap()
    bdmask = nc.dram_tensor("bdmask", (CH, W17), f32, kind="ExternalInput").ap()
    hmask = nc.dram_tensor("hmask", (CH, HEADS), f32, kind="ExternalInput").ap()
    y8 = nc.dram_tensor("y8", (L * n_seq, CH), i8, kind="ExternalOutput").ap()
    osc = nc.dram_tensor("osc", (C, ncol), f16, kind="ExternalOutput").ap()

    x3 = xq.rearrange("(t n) f -> t n f", n=n_seq)
    y3 = y8.rearrange("(t n) f -> t n f", n=n_seq)

    with tile.TileContext(nc) as tc, ExitStack() as ctx:
        cpool = ctx.enter_context(tc.tile_pool(name="consts", bufs=1))
        wq_sb = cpool.tile([F, CH], f16, tag="wq")
        wk_sb = cpool.tile([F, CH], f16, tag="wk")
        wv_sb = cpool.tile([F, CH], f16, tag="wv")
        nc.scalar.dma_start(wq_sb[:], wq)
        nc.scalar.dma_start(wk_sb[:], wk)
        nc.scalar.dma_start(wv_sb[:], wv)
        bq_sb = cpool.tile([1, CH], f16, tag="bq")
        bk_sb = cpool.tile([1, CH], f16, tag="bk")
        bv_sb = cpool.tile([1, CH], f16, tag="bv")
        nc.scalar.dma_start(bq_sb[:], bq.rearrange("(a f) -> a f", a=1))
        nc.scalar.dma_start(bk_sb[:], bk.rearrange("(a f) -> a f", a=1))
        nc.scalar.dma_start(bv_sb[:], bv.rearrange("(a f) -> a f", a=1))
        ones_sb = cpool.tile([1, C], f16, tag="ones")
        nc.vector.memset(ones_sb[:], 1.0)
        cm_sb = cpool.tile([C, C], f16, tag="cmask")
        nc.scalar.dma_start(cm_sb[:], cmask)
        bd_sb = cpool.tile([CH, W17], f32, tag="bdmask")
        nc.scalar.dma_start(bd_sb[:], bdmask)
        hm_sb = cpool.tile([CH, HEADS], f32, tag="hmask")
        nc.scalar.dma_start(hm_sb[:], hmask)
        xsc_sb = cpool.tile([C, ncol], f32, tag="xsc")
        nc.scalar.dma_start(xsc_sb[:], xsc)
        osc_sb = cpool.tile([C, ncol], f16, tag="osc")

        xpool = ctx.enter_context(tc.tile_pool(name="x", bufs=3))
        phipool = ctx.enter_context(tc.tile_pool(name="phi", bufs=3))
        spool = ctx.enter_context(tc.tile_pool(name="sacc", bufs=1))
        tpool = ctx.enter_context(tc.tile_pool(name="tmp", bufs=2))
        opool = ctx.enter_context(tc.tile_pool(name="out", bufs=3))
        ps_proj = ctx.enter_context(tc.tile_pool(name="psproj", bufs=4, space="PSUM"))
        ps_at = ctx.enter_context(tc.tile_pool(name="psat", bufs=1, space="PSUM"))
        ps_acc = ctx.enter_context(tc.tile_pool(name="psacc", bufs=1, space="PSUM"))
        ps_inta = ctx.enter_context(tc.tile_pool(name="psinta", bufs=1, space="PSUM"))
        ps_g = ctx.enter_context(tc.tile_pool(name="psg", bufs=1, space="PSUM"))

        def phi(dst, ps):
            # phi(x) = elu(x) + 1 = relu(x) + exp(min(x, 0))
            shape = [ps.shape[0], ps.shape[1]]
            a = tpool.tile(shape, f32, tag="phia")
            b = tpool.tile(shape, f32, tag="phib")
            nc.scalar.activation(a[:], ps[:], Relu)
            nc.vector.tensor_scalar_min(b[:], ps[:], 0.0)
            nc.scalar.activation(b[:], b[:], Exp)
            nc.vector.tensor_add(dst[:], a[:], b[:])

        for n in range(n_seq):
            S_acc = spool.tile([CH, W17], f32, tag="sacc")
            nc.vector.memset(S_acc[:], 0.0)
            for c in range(NC):
                col = n * NC + c
                # load int8 chunk [tok, F], dequant per-token, transpose to [F, tok]
                xi8 = xpool.tile([C, F], i8, tag="xi8")
                nc.scalar.dma_start(xi8[:], x3[c * C:(c + 1) * C, n, :])
                x16 = xpool.tile([C, F], f16, tag="x16")
                nc.vector.tensor_scalar_mul(x16[:], xi8[:], xsc_sb[:, col:col + 1])
                xT = xpool.tile([F, C], f16, tag="xT")
                nc.sync.dma_start(xT[:], x16[:], transpose=True)

                # projections (+ rank-1 bias add)
                qT_ps = ps_proj.tile([CH, C], f32, tag="proj")
                kT_ps = ps_proj.tile([CH, C], f32, tag="proj")
                kt_ps = ps_proj.tile([C, CH], f32, tag="proj")
                vt_ps = ps_proj.tile([C, CH], f32, tag="proj")
                nc.tensor.matmul(qT_ps[:], wq_sb[:], xT[:], start=True, stop=False)
                nc.tensor.matmul(qT_ps[:], bq_sb[:], ones_sb[:], start=False, stop=True)
                nc.tensor.matmul(kT_ps[:], wk_sb[:], xT[:], start=True, stop=False)
                nc.tensor.matmul(kT_ps[:], bk_sb[:], ones_sb[:], start=False, stop=True)
                nc.tensor.matmul(kt_ps[:], xT[:], wk_sb[:], start=True, stop=False)
                nc.tensor.matmul(kt_ps[:], ones_sb[:], bk_sb[:], start=False, stop=True)
                nc.tensor.matmul(vt_ps[:], xT[:], wv_sb[:], start=True, stop=False)
                nc.tensor.matmul(vt_ps[:], ones_sb[:], bv_sb[:], start=False, stop=True)

                q16 = phipool.tile([CH, C], f16, tag="q16")   # phi(q)^T [chan, tok]
                k16 = phipool.tile([CH, C], f16, tag="k16")   # phi(k)^T [chan, tok]
                kt16 = phipool.tile([C, CH], f16, tag="kt16")  # phi(k) [tok, chan]
                phi(q16, qT_ps)
                phi(k16, kT_ps)
                phi(kt16, kt_ps)

                # v_aug [tok, 136]: per head h cols h*17..h*17+15 = v_h, col h*17+16 = 1
                vaug = phipool.tile([C, W17], f16, tag="vaug")
                va = vaug[:].rearrange("p (h j) -> p h j", j=17)
                vs = vt_ps[:].rearrange("p (h j) -> p h j", j=16)
                nc.vector.tensor_copy(va[:, :, 0:16], vs)
                nc.vector.memset(va[:, :, 16:17], 1.0)

                # inter-chunk: acc[t, :] = phi(q)_t @ S_prev (block-diag interleaved)
                s16 = phipool.tile([CH, W17], f16, tag="s16")
                nc.vector.tensor_copy(s16[:], S_acc[:])
                acc_ps = ps_acc.tile([C, W17], f32, tag="acc")
                nc.tensor.matmul(acc_ps[:], q16[:], s16[:], start=True, stop=True)

                # intra-chunk per head: A^T = (k.head_mask)^T q (K=128, head-
                # masked k zeroes cross-head terms), mask causal, A_m^T.T@[v|1]
                inta_ps = ps_inta.tile([C, W17], f32, tag="inta")
                for h in range(HEADS):
                    kh = tpool.tile([CH, C], f16, tag="kh")
                    nc.vector.tensor_scalar_mul(kh[:], k16[:], hm_sb[:, h:h + 1])
                    at_ps = ps_at.tile([C, C], f32, tag="at")
                    nc.tensor.matmul(
                        at_ps[:], kh[:], q16[:], start=True, stop=True,
                    )
                    am = tpool.tile([C, C], f16, tag="am")
                    nc.vector.tensor_mul(am[:], at_ps[:], cm_sb[:])
                    nc.tensor.matmul(
                        inta_ps[:, h * 17:h * 17 + 17],
                        am[:],
                        vaug[:, h * 17:h * 17 + 17],
                        start=True, stop=True,
                    )

                # KV gram for this chunk + masked accumulate into S
                g_ps = ps_g.tile([CH, W17], f32, tag="g")
                nc.tensor.matmul(g_ps[:], kt16[:], vaug[:], start=True, stop=True)
                gt = tpool.tile([CH, W17], f32, tag="gt")
                nc.vector.tensor_mul(gt[:], g_ps[:], bd_sb[:])
                nc.vector.tensor_add(S_acc[:], S_acc[:], gt[:])

                # normalize: out = (num_inter + num_intra) / (den_i + den_x + eps)
                # DVE reads at most one PSUM operand: stage intra to SBUF first.
                inta_sb = tpool.tile([C, W17], f32, tag="intasb")
                nc.vector.tensor_copy(inta_sb[:], inta_ps[:])
                accv = acc_ps[:].rearrange("p (h j) -> p h j", j=17)
                intav = inta_sb[:].rearrange("p (h j) -> p h j", j=17)
                den = tpool.tile([C, HEADS], f32, tag="den")
                dv = den[:].rearrange("p (h j) -> p h j", j=1)
                nc.vector.scalar_tensor_tensor(
                    dv, accv[:, :, 16:17], EPS, intav[:, :, 16:17],
                    op0=AluOp.add, op1=AluOp.add,
                )
                rec = tpool.tile([C, HEADS], f32, tag="rec")
                nc.vector.reciprocal(rec[:], den[:])
                out_f = opool.tile([C, CH], f32, tag="outf")
                for h in range(HEADS):
                    nsum = tpool.tile([C, E], f32, tag="nsum")
                    nc.vector.tensor_add(
                        nsum[:],
                        acc_ps[:, h * 17:h * 17 + 16],
                        inta_sb[:, h * 17:h * 17 + 16],
                    )
                    nc.vector.tensor_scalar_mul(
                        out_f[:, h * 16:(h + 1) * 16],
                        nsum[:],
                        rec[:, h:h + 1],
                    )

                # int8 quantize per token: amax, scale out, store scale
                amax = tpool.tile([C, 1], f32, tag="amax")
                nc.vector.reduce_max(
                    amax[:], out_f[:], axis=AX, apply_absolute_value=True
                )
                nc.vector.tensor_scalar_max(amax[:], amax[:], AMAX_FLOOR)
                nc.vector.tensor_scalar_mul(
                    osc_sb[:, col:col + 1], amax[:], 1.0 / 127.0
                )
                r8 = tpool.tile([C, 1], f32, tag="r8")
                nc.vector.reciprocal(r8[:], amax[:])
                y8t = opool.tile([C, CH], i8, tag="y8t")
                nc.vector.tensor_scalar(
                    y8t[:], out_f[:], r8[:, 0:1], 127.0,
                    op0=AluOp.mult, op1=AluOp.mult,
                )
                nc.gpsimd.dma_start(y3[c * C:(c + 1) * C, n, :], y8t[:])

        nc.gpsimd.dma_start(osc, osc_sb[:])

    nc.compile()
    return nc


def _consts():
    cmask = np.triu(np.ones((C, C), np.float16))  # cmask[s,t] = 1 if s<=t
    bd = np.zeros((CH, W17), np.float32)
    for h in range(HEADS):
        bd[h * 16:(h + 1) * 16, h * 17:(h + 1) * 17] = 1.0
    hm = np.zeros((CH, HEADS), np.float32)
    for h in range(HEADS):
        hm[h * 16:(h + 1) * 16, h] = 1.0
    return cmask, bd, hm


def quantize_input(q2d):
    """q2d: (rows, 128) fp32 -> int8 rows + per-(n,c)-tile scale layout."""
    amax = np.abs(q2d).max(axis=1)
    np.maximum(amax, 1e-12, out=amax)
    sc = amax * (1.0 / 127.0)
    xi8 = np.round(q2d * (1.0 / sc)[:, None]).astype(np.int8)
    return xi8, sc


_RUNNER = None


def _make_runner():
    import jax
    from jax.sharding import Mesh, NamedSharding, PartitionSpec

    try:
        from jax.experimental.shard_map import shard_map
    except ImportError:
        from jax.shard_map import shard_map

    import concourse.mybir as mybir
    from concourse.bass2jax import (
        _bass_exec_p,
        install_neuronx_cc_hook,
        partition_id_tensor,
    )

    install_neuronx_cc_hook()
    nc = build_nc(NSEQ)

    partition_name = (
        nc.partition_id_tensor.name if nc.partition_id_tensor is not None else None
    )
    in_names: list[str] = []
    out_names: list[str] = []
    out_avals = []
    zero_outs = []
    for alloc in nc.m.functions[0].allocations:
        if not isinstance(alloc, mybir.MemoryLocationSet):
            continue
        name = alloc.memorylocations[0].name
        if alloc.kind == "ExternalInput":
            if name != partition_name:
                in_names.append(name)
        elif alloc.kind == "ExternalOutput":
            out_names.append(name)
            shape = tuple(alloc.tensor_shape)
            dtype = mybir.dt.np(alloc.dtype)
            out_avals.append(jax.core.ShapedArray(shape, dtype))
            zero_outs.append(np.zeros((8 * shape[0], *shape[1:]), dtype))
    n_params = len(in_names)
    all_in_names = in_names + out_names
    if partition_name is not None:
        all_in_names = all_in_names + [partition_name]

    def _body(*args):
        operands = list(args)
        if partition_name is not None:
            operands.append(partition_id_tensor())
        outs = _bass_exec_p.bind(
            *operands,
            out_avals=tuple(out_avals),
            in_names=tuple(all_in_names),
            out_names=tuple(out_names),
            lowering_input_output_aliases=(),
            sim_require_finite=True,
            sim_require_nnan=True,
            nc=nc,
        )
        return tuple(outs)

    devices = jax.devices()[:8]
    mesh = Mesh(np.asarray(devices), ("core",))
    spec = NamedSharding(mesh, PartitionSpec("core"))
    nin = n_params + len(out_names)
    sharded = jax.jit(
        shard_map(
            _body,
            mesh=mesh,
            in_specs=(PartitionSpec("core"),) * nin,
            out_specs=(PartitionSpec("core"),) * len(out_names),
            check_rep=False,
        ),
        keep_unused=True,
    )

    cmask, bd, hm = _consts()
    persist = {
        "cmask": jax.device_put(np.tile(cmask, (8, 1)), spec),
        "bdmask": jax.device_put(np.tile(bd, (8, 1)), spec),
        "hmask": jax.device_put(np.tile(hm, (8, 1)), spec),
    }
    zeros_dev = [jax.device_put(z, spec) for z in zero_outs]
    oi = {nm: i for i, nm in enumerate(out_names)}

    import concurrent.futures as cf

    pool = cf.ThreadPoolExecutor(8)

    def run(query, Wq, bq_, Wk, bk_, Wv, bv_):
        # threaded per-core int8 quantization of the input
        q2d = query.reshape(8, L * NSEQ, F)

        def quant_core(b):
            return quantize_input(q2d[b])

        parts = list(pool.map(quant_core, range(8)))
        xi8 = np.concatenate([p[0] for p in parts], axis=0)
        # scale layout per core: [C (j), NSEQ*NC (n*NC+c)], token (c*C+j)*NSEQ+n
        xsc = np.empty((8 * C, NSEQ * NC), np.float32)
        for b in range(8):
            s = parts[b][1].reshape(NC, C, NSEQ)  # [(c, j), n]
            xsc[b * C:(b + 1) * C] = (
                s.transpose(1, 2, 0).reshape(C, NSEQ * NC)
            )
        vals = {
            "xq": xi8,
            "xsc": xsc,
            "wq": np.tile(np.asarray(Wq, np.float16), (8, 1)),
            "wk": np.tile(np.asarray(Wk, np.float16), (8, 1)),
            "wv": np.tile(np.asarray(Wv, np.float16), (8, 1)),
            "bq": np.tile(np.asarray(bq_, np.float16), 8),
            "bk": np.tile(np.asarray(bk_, np.float16), 8),
            "bv": np.tile(np.asarray(bv_, np.float16), 8),
        }
        args = [persist.get(nm) if nm in persist else vals[nm] for nm in in_names]
        out = sharded(*args, *zeros_dev)
        y8g = out[oi["y8"]]
        oscg = out[oi["osc"]]
        y8 = np.asarray(y8g).reshape(8, L * NSEQ, CH)
        oscs = np.asarray(oscg, np.float32).reshape(8, C, NSEQ * NC)

        # threaded dequantization: y = y8 * osc[token]
        res = np.empty((8, L * NSEQ, CH), np.float32)

        def dequant_core(b):
            s = oscs[b].reshape(C, NSEQ, NC).transpose(2, 0, 1)  # [c, j, n]
            sflat = s.reshape(L, NSEQ).reshape(L * NSEQ)
            res[b] = y8[b].astype(np.float32) * sflat[:, None]

        list(pool.map(dequant_core, range(8)))
        return res.reshape(8, L, NSEQ, CH)

    return run


def kernel(query, Wq, bq, Wk, bk, Wv, bv):
    global _RUNNER
    if _RUNNER is None:
        _RUNNER = _make_runner()
    return _RUNNER(np.asarray(query, np.float32), Wq, bq, Wk, bk, Wv, bv)



# revision 5
# speedup vs baseline: 1.2745x; 1.2745x over previous
"""Causal linear attention (fast_transformers style) on 8 Trainium2 cores.

query (8, 512, 64, 128) f32. Data-parallel: one batch element per core.
Per (batch, node) sequence of L=512 tokens: project q/k/v with 128x128
weights, phi(x)=elu(x)+1, causal linear attention via chunked scan
(C=128 intra-chunk masked matmul + inter-chunk running KV state).

Wire strategy: the axon tunnel is half-duplex at ~40-45 MiB/s and is the
whole cost, so ship int8 with per-token scales both directions. All
per-call inputs (int8 x, f16 scales, f16 weights/biases) are packed into
ONE int8 blob per core (bitcast regions on device) so each direction is
a single large transfer; the node axis is split into S slices pipelined
as separate async calls so host quant/dequant and device exec hide under
the serialized wire time. Matmuls run fp16 with fp32 PSUM accumulation;
mask constants and output zero-buffers stay resident on device.
"""

import numpy as np

HEADS = 8
E = 16
EPS = 1e-6
L = 512
NSEQ = 64
F = 128
CH = HEADS * E  # 128 output channels
C = 128         # time chunk
NC = L // C
W17 = 17 * HEADS  # 136: per-head [num(16) | den(1)] interleaved width
AMAX_FLOOR = 1e-4

S = 2                  # node-axis slices pipelined per kernel() call
NS = NSEQ // S         # nodes per core per slice-call
NCOL = NS * NC         # scale columns per slice: one per (n, c) tile

# int8 blob row layout (rows x 128 bytes), input side
R_X = L * NS                         # xq int8 rows, row = t*NS + n
R_SC = C * NCOL * 2 // F             # f16 scales region rows
R_W = (F * CH * 2) // F              # one f16 weight matrix region rows
R_B = (3 * CH * 2 + F - 1) // F      # three f16 biases region rows
R_IN = R_X + R_SC + 3 * R_W + R_B
R_OUT = R_X + R_SC                   # y8 rows + f16 out-scales rows


def build_nc(debug=False):
    """Build the per-core Bass module for one slice-call (NS node seqs)."""
    from contextlib import ExitStack

    import concourse.bacc as bacc
    import concourse.mybir as mybir
    import concourse.tile as tile

    i8 = mybir.dt.int8
    f16 = mybir.dt.float16
    f32 = mybir.dt.float32
    Relu = mybir.ActivationFunctionType.Relu
    Exp = mybir.ActivationFunctionType.Exp
    AluOp = mybir.AluOpType
    AX = mybir.AxisListType.X

    nc = bacc.Bacc(
        "TRN2",
        target_bir_lowering=False,
        debug=debug,
        enable_asserts=False,
        num_devices=8,
    )

    blob = nc.dram_tensor("blob", (R_IN, F), i8, kind="ExternalInput").ap()
    cmask = nc.dram_tensor("cmask", (C, C), f16, kind="ExternalInput").ap()
    bdmask = nc.dram_tensor("bdmask", (CH, W17), f32, kind="ExternalInput").ap()
    hmask = nc.dram_tensor("hmask", (CH, HEADS), f32, kind="ExternalInput").ap()
    yout = nc.dram_tensor("yout", (R_OUT, F), i8, kind="ExternalOutput").ap()

    o = R_X
    xsc = blob[o:o + R_SC, :].bitcast(f16).rearrange("(a b) c -> a (b c)", b=2)
    o += R_SC
    wq = blob[o:o + R_W, :].bitcast(f16).rearrange("(a b) c -> a (b c)", b=2)
    o += R_W
    wk = blob[o:o + R_W, :].bitcast(f16).rearrange("(a b) c -> a (b c)", b=2)
    o += R_W
    wv = blob[o:o + R_W, :].bitcast(f16).rearrange("(a b) c -> a (b c)", b=2)
    o += R_W
    bqkv = blob[o:o + R_B, :].bitcast(f16).rearrange("(a b) c -> a (b c)", b=2)

    x3 = blob[0:R_X, :].rearrange("(t n) f -> t n f", n=NS)
    y3 = yout[0:R_X, :].rearrange("(t n) f -> t n f", n=NS)
    osc_out = yout[R_X:R_OUT, :].bitcast(f16).rearrange("(a b) c -> a (b c)", b=2)

    with tile.TileContext(nc) as tc, ExitStack() as ctx:
        cpool = ctx.enter_context(tc.tile_pool(name="consts", bufs=1))
        wq_sb = cpool.tile([F, CH], f16, tag="wq")
        wk_sb = cpool.tile([F, CH], f16, tag="wk")
        wv_sb = cpool.tile([F, CH], f16, tag="wv")
        nc.scalar.dma_start(wq_sb[:], wq)
        nc.scalar.dma_start(wk_sb[:], wk)
        nc.scalar.dma_start(wv_sb[:], wv)
        bq_sb = cpool.tile([1, CH], f16, tag="bq")
        bk_sb = cpool.tile([1, CH], f16, tag="bk")
        bv_sb = cpool.tile([1, CH], f16, tag="bv")
        nc.scalar.dma_start(bq_sb[:], bqkv[0:1, :])
        nc.scalar.dma_start(bk_sb[:], bqkv[1:2, :])
        nc.scalar.dma_start(bv_sb[:], bqkv[2:3, :])
        ones_sb = cpool.tile([1, C], f16, tag="ones")
        nc.vector.memset(ones_sb[:], 1.0)
        cm_sb = cpool.tile([C, C], f16, tag="cmask")
        nc.scalar.dma_start(cm_sb[:], cmask)
        bd_sb = cpool.tile([CH, W17], f32, tag="bdmask")
        nc.scalar.dma_start(bd_sb[:], bdmask)
        hm_sb = cpool.tile([CH, HEADS], f32, tag="hmask")
        nc.scalar.dma_start(hm_sb[:], hmask)
        xsc16 = cpool.tile([C, NCOL], f16, tag="xsc16")
        nc.scalar.dma_start(xsc16[:], xsc)
        xsc_sb = cpool.tile([C, NCOL], f32, tag="xsc")
        nc.vector.tensor_copy(xsc_sb[:], xsc16[:])
        osc_sb = cpool.tile([C, NCOL], f16, tag="osc")

        xpool = ctx.enter_context(tc.tile_pool(name="x", bufs=3))
        phipool = ctx.enter_context(tc.tile_pool(name="phi", bufs=3))
        spool = ctx.enter_context(tc.tile_pool(name="sacc", bufs=1))
        tpool = ctx.enter_context(tc.tile_pool(name="tmp", bufs=2))
        opool = ctx.enter_context(tc.tile_pool(name="out", bufs=3))
        ps_proj = ctx.enter_context(tc.tile_pool(name="psproj", bufs=4, space="PSUM"))
        ps_at = ctx.enter_context(tc.tile_pool(name="psat", bufs=1, space="PSUM"))
        ps_acc = ctx.enter_context(tc.tile_pool(name="psacc", bufs=1, space="PSUM"))
        ps_inta = ctx.enter_context(tc.tile_pool(name="psinta", bufs=1, space="PSUM"))
        ps_g = ctx.enter_context(tc.tile_pool(name="psg", bufs=1, space="PSUM"))

        def phi(dst, ps):
            # phi(x) = elu(x) + 1 = relu(x) + exp(min(x, 0))
            shape = [ps.shape[0], ps.shape[1]]
            a = tpool.tile(shape, f32, tag="phia")
            b = tpool.tile(shape, f32, tag="phib")
            nc.scalar.activation(a[:], ps[:], Relu)
            nc.vector.tensor_scalar_min(b[:], ps[:], 0.0)
            nc.scalar.activation(b[:], b[:], Exp)
            nc.vector.tensor_add(dst[:], a[:], b[:])

        for n in range(NS):
            S_acc = spool.tile([CH, W17], f32, tag="sacc")
            nc.vector.memset(S_acc[:], 0.0)
            for c in range(NC):
                col = n * NC + c
                # load int8 chunk [tok, F], dequant per-token, transpose to [F, tok]
                xi8 = xpool.tile([C, F], i8, tag="xi8")
                nc.scalar.dma_start(xi8[:], x3[c * C:(c + 1) * C, n, :])
                x16 = xpool.tile([C, F], f16, tag="x16")
                nc.vector.tensor_scalar_mul(x16[:], xi8[:], xsc_sb[:, col:col + 1])
                xT = xpool.tile([F, C], f16, tag="xT")
                nc.sync.dma_start(xT[:], x16[:], transpose=True)

                # projections (+ rank-1 bias add)
                qT_ps = ps_proj.tile([CH, C], f32, tag="proj")
                kT_ps = ps_proj.tile([CH, C], f32, tag="proj")
                kt_ps = ps_proj.tile([C, CH], f32, tag="proj")
                vt_ps = ps_proj.tile([C, CH], f32, tag="proj")
                nc.tensor.matmul(qT_ps[:], wq_sb[:], xT[:], start=True, stop=False)
                nc.tensor.matmul(qT_ps[:], bq_sb[:], ones_sb[:], start=False, stop=True)
                nc.tensor.matmul(kT_ps[:], wk_sb[:], xT[:], start=True, stop=False)
                nc.tensor.matmul(kT_ps[:], bk_sb[:], ones_sb[:], start=False, stop=True)
                nc.tensor.matmul(kt_ps[:], xT[:], wk_sb[:], start=True, stop=False)
                nc.tensor.matmul(kt_ps[:], ones_sb[:], bk_sb[:], start=False, stop=True)
                nc.tensor.matmul(vt_ps[:], xT[:], wv_sb[:], start=True, stop=False)
                nc.tensor.matmul(vt_ps[:], ones_sb[:], bv_sb[:], start=False, stop=True)

                q16 = phipool.tile([CH, C], f16, tag="q16")   # phi(q)^T [chan, tok]
                k16 = phipool.tile([CH, C], f16, tag="k16")   # phi(k)^T [chan, tok]
                kt16 = phipool.tile([C, CH], f16, tag="kt16")  # phi(k) [tok, chan]
                phi(q16, qT_ps)
                phi(k16, kT_ps)
                phi(kt16, kt_ps)

                # v_aug [tok, 136]: per head h cols h*17..h*17+15 = v_h, col h*17+16 = 1
                vaug = phipool.tile([C, W17], f16, tag="vaug")
                va = vaug[:].rearrange("p (h j) -> p h j", j=17)
                vs = vt_ps[:].rearrange("p (h j) -> p h j", j=16)
                nc.vector.tensor_copy(va[:, :, 0:16], vs)
                nc.vector.memset(va[:, :, 16:17], 1.0)

                # inter-chunk: acc[t, :] = phi(q)_t @ S_prev (block-diag interleaved)
                s16 = phipool.tile([CH, W17], f16, tag="s16")
                nc.vector.tensor_copy(s16[:], S_acc[:])
                acc_ps = ps_acc.tile([C, W17], f32, tag="acc")
                nc.tensor.matmul(acc_ps[:], q16[:], s16[:], start=True, stop=True)

                # intra-chunk per head: A^T = (k.head_mask)^T q (K=128, head-
                # masked k zeroes cross-head terms), mask causal, A_m^T.T@[v|1]
                inta_ps = ps_inta.tile([C, W17], f32, tag="inta")
                for h in range(HEADS):
                    kh = tpool.tile([CH, C], f16, tag="kh")
                    nc.vector.tensor_scalar_mul(kh[:], k16[:], hm_sb[:, h:h + 1])
                    at_ps = ps_at.tile([C, C], f32, tag="at")
                    nc.tensor.matmul(
                        at_ps[:], kh[:], q16[:], start=True, stop=True,
                    )
                    am = tpool.tile([C, C], f16, tag="am")
                    nc.vector.tensor_mul(am[:], at_ps[:], cm_sb[:])
                    nc.tensor.matmul(
                        inta_ps[:, h * 17:h * 17 + 17],
                        am[:],
                        vaug[:, h * 17:h * 17 + 17],
                        start=True, stop=True,
                    )

                # KV gram for this chunk + masked accumulate into S
                g_ps = ps_g.tile([CH, W17], f32, tag="g")
                nc.tensor.matmul(g_ps[:], kt16[:], vaug[:], start=True, stop=True)
                gt = tpool.tile([CH, W17], f32, tag="gt")
                nc.vector.tensor_mul(gt[:], g_ps[:], bd_sb[:])
                nc.vector.tensor_add(S_acc[:], S_acc[:], gt[:])

                # normalize: out = (num_inter + num_intra) / (den_i + den_x + eps)
                # DVE reads at most one PSUM operand: stage intra to SBUF first.
                inta_sb = tpool.tile([C, W17], f32, tag="intasb")
                nc.vector.tensor_copy(inta_sb[:], inta_ps[:])
                accv = acc_ps[:].rearrange("p (h j) -> p h j", j=17)
                intav = inta_sb[:].rearrange("p (h j) -> p h j", j=17)
                den = tpool.tile([C, HEADS], f32, tag="den")
                dv = den[:].rearrange("p (h j) -> p h j", j=1)
                nc.vector.scalar_tensor_tensor(
                    dv, accv[:, :, 16:17], EPS, intav[:, :, 16:17],
                    op0=AluOp.add, op1=AluOp.add,
                )
                rec = tpool.tile([C, HEADS], f32, tag="rec")
                nc.vector.reciprocal(rec[:], den[:])
                out_f = opool.tile([C, CH], f32, tag="outf")
                for h in range(HEADS):
                    nsum = tpool.tile([C, E], f32, tag="nsum")
                    nc.vector.tensor_add(
                        nsum[:],
                        acc_ps[:, h * 17:h * 17 + 16],
                        inta_sb[:, h * 17:h * 17 + 16],
                    )
                    nc.vector.tensor_scalar_mul(
                        out_f[:, h * 16:(h + 1) * 16],
                        nsum[:],
                        rec[:, h:h + 1],
                    )

                # int8 quantize per token: amax, scale out, store scale
                amax = tpool.tile([C, 1], f32, tag="amax")
                nc.vector.reduce_max(
                    amax[:], out_f[:], axis=AX, apply_absolute_value=True
                )
                nc.vector.tensor_scalar_max(amax[:], amax[:], AMAX_FLOOR)
                nc.vector.tensor_scalar_mul(
                    osc_sb[:, col:col + 1], amax[:], 1.0 / 127.0
                )
                r8 = tpool.tile([C, 1], f32, tag="r8")
                nc.vector.reciprocal(r8[:], amax[:])
                y8t = opool.tile([C, CH], i8, tag="y8t")
                nc.vector.tensor_scalar(
                    y8t[:], out_f[:], r8[:, 0:1], 127.0,
                    op0=AluOp.mult, op1=AluOp.mult,
                )
                nc.gpsimd.dma_start(y3[c * C:(c + 1) * C, n, :], y8t[:])

        nc.gpsimd.dma_start(osc_out, osc_sb[:])

    nc.compile()
    return nc


def _consts():
    cmask = np.triu(np.ones((C, C), np.float16))  # cmask[s,t] = 1 if s<=t
    bd = np.zeros((CH, W17), np.float32)
    for h in range(HEADS):
        bd[h * 16:(h + 1) * 16, h * 17:(h + 1) * 17] = 1.0
    hm = np.zeros((CH, HEADS), np.float32)
    for h in range(HEADS):
        hm[h * 16:(h + 1) * 16, h] = 1.0
    return cmask, bd, hm


_RUNNER = None


def _make_runner():
    import jax
    from jax.sharding import Mesh, NamedSharding, PartitionSpec

    try:
        from jax.experimental.shard_map import shard_map
    except ImportError:
        from jax.shard_map import shard_map

    import concourse.mybir as mybir
    from concourse.bass2jax import (
        _bass_exec_p,
        install_neuronx_cc_hook,
        partition_id_tensor,
    )

    install_neuronx_cc_hook()
    nc = build_nc()

    partition_name = (
        nc.partition_id_tensor.name if nc.partition_id_tensor is not None else None
    )
    in_names: list[str] = []
    out_names: list[str] = []
    out_avals = []
    zero_outs = []
    for alloc in nc.m.functions[0].allocations:
        if not isinstance(alloc, mybir.MemoryLocationSet):
            continue
        name = alloc.memorylocations[0].name
        if alloc.kind == "ExternalInput":
            if name != partition_name:
                in_names.append(name)
        elif alloc.kind == "ExternalOutput":
            out_names.append(name)
            shape = tuple(alloc.tensor_shape)
            dtype = mybir.dt.np(alloc.dtype)
            out_avals.append(jax.core.ShapedArray(shape, dtype))
            zero_outs.append(np.zeros((8 * shape[0], *shape[1:]), dtype))
    n_params = len(in_names)
    all_in_names = in_names + out_names
    if partition_name is not None:
        all_in_names = all_in_names + [partition_name]

    def _body(*args):
        operands = list(args)
        if partition_name is not None:
            operands.append(partition_id_tensor())
        outs = _bass_exec_p.bind(
            *operands,
            out_avals=tuple(out_avals),
            in_names=tuple(all_in_names),
            out_names=tuple(out_names),
            lowering_input_output_aliases=(),
            sim_require_finite=True,
            sim_require_nnan=True,
            nc=nc,
        )
        return tuple(outs)

    devices = jax.devices()[:8]
    mesh = Mesh(np.asarray(devices), ("core",))
    spec = NamedSharding(mesh, PartitionSpec("core"))
    nin = n_params + len(out_names)
    sharded = jax.jit(
        shard_map(
            _body,
            mesh=mesh,
            in_specs=(PartitionSpec("core"),) * nin,
            out_specs=(PartitionSpec("core"),) * len(out_names),
            check_rep=False,
        ),
        keep_unused=True,
    )

    cmask, bd, hm = _consts()
    persist = {
        "cmask": jax.device_put(np.tile(cmask, (8, 1)), spec),
        "bdmask": jax.device_put(np.tile(bd, (8, 1)), spec),
        "hmask": jax.device_put(np.tile(hm, (8, 1)), spec),
    }
    zeros_dev = [jax.device_put(z, spec) for z in zero_outs]
    oi = {nm: i for i, nm in enumerate(out_names)}
    yi = oi["yout"]

    # persistent per-slice input blobs (numpy, reused across calls)
    blobs = [np.empty((8 * R_IN, F), np.int8) for _ in range(S)]

    O_SC = R_X
    O_WQ = O_SC + R_SC
    O_WK = O_WQ + R_W
    O_WV = O_WK + R_W
    O_B = O_WV + R_W

    def pack_slice(s, query, wbytes):
        """Quantize node-slice s of all 8 batches into blobs[s]."""
        blob = blobs[s]
        n0 = s * NS
        for b in range(8):
            base = b * R_IN
            xs = query[b, :, n0:n0 + NS, :]  # (L, NS, F) f32 view
            amax = np.maximum(xs.max(axis=2), -xs.min(axis=2))  # (L, NS)
            np.maximum(amax, 1e-12, out=amax)
            inv = np.float32(127.0) / amax
            q = xs * inv[:, :, None]
            np.rint(q, out=q)
            xv = blob[base:base + R_X, :].reshape(L, NS, F)
            np.copyto(xv, q, casting="unsafe")
            # scales f16, layout [j, n*NC+c] from amax[t= c*C+j, n]
            sc = amax * np.float32(1.0 / 127.0)
            sct = sc.reshape(NC, C, NS).transpose(1, 2, 0).reshape(C, NCOL)
            scv = blob[base + O_SC:base + O_WQ, :].view(np.float16)
            scv.reshape(C, NCOL)[:] = sct
            blob[base + O_WQ:base + O_B + R_B, :] = wbytes
        return blob

    def unpack_slice(s, y, res):
        """Dequantize slice-call output y into res[:, :, n0:n1, :]."""
        n0 = s * NS
        for b in range(8):
            base = b * R_OUT
            yb = y[base:base + R_X, :].reshape(L, NS, CH)
            oscb = y[base + R_X:base + R_OUT, :].view(np.float16)
            sf = (
                oscb.reshape(C, NS, NC)
                .transpose(2, 0, 1)
                .reshape(L, NS)
                .astype(np.float32)
            )
            np.multiply(yb, sf[:, :, None], out=res[b, :, n0:n0 + NS, :])

    def run(query, Wq, bq_, Wk, bk_, Wv, bv_):
        query = np.ascontiguousarray(np.asarray(query, np.float32))
        # weight+bias byte block, shared by every core/slice
        wbytes = np.empty((3 * R_W + R_B, F), np.int8)
        wv16 = wbytes[0:3 * R_W, :].view(np.float16).reshape(3, F, CH)
        wv16[0] = Wq
        wv16[1] = Wk
        wv16[2] = Wv
        bv16 = wbytes[3 * R_W:, :].view(np.float16).reshape(3, CH)
        bv16[0] = bq_
        bv16[1] = bk_
        bv16[2] = bv_

        outs = []
        for s in range(S):
            blob = pack_slice(s, query, wbytes)
            out = sharded(blob, *(persist[nm] for nm in in_names[1:]), *zeros_dev)
            y = out[yi]
            y.copy_to_host_async()
            outs.append(y)

        res = np.empty((8, L, NSEQ, CH), np.float32)
        for s in range(S):
            y = np.asarray(outs[s])
            unpack_slice(s, y, res)
        return res

    # in_names sanity: blob must be first, rest must be persisted consts
    assert in_names[0] == "blob", in_names
    assert all(nm in persist for nm in in_names[1:]), in_names

    return run


def kernel(query, Wq, bq, Wk, bk, Wv, bv):
    global _RUNNER
    if _RUNNER is None:
        _RUNNER = _make_runner()
    return _RUNNER(np.asarray(query, np.float32), Wq, bq, Wk, bk, Wv, bv)


# revision 6
# speedup vs baseline: 1.3288x; 1.0426x over previous
"""Causal linear attention (fast_transformers style) on 8 Trainium2 cores.

query (8, 512, 64, 128) f32. Data-parallel: one batch element per core.
Per (batch, node) sequence of L=512 tokens: project q/k/v with 128x128
weights, phi(x)=elu(x)+1, causal linear attention via chunked scan
(C=128 intra-chunk masked matmul + inter-chunk running KV state).

Wire strategy: the axon tunnel is half-duplex at ~40-45 MiB/s and is the
whole cost, so ship int8 with per-token scales both directions. All
per-call inputs (int8 x, f16 scales, f16 weights/biases) are packed into
ONE int8 blob per core (bitcast regions on device) so each direction is
a single large transfer; the node axis is split into S slices pipelined
as separate async calls so host quant/dequant and device exec hide under
the serialized wire time. Matmuls run fp16 with fp32 PSUM accumulation;
mask constants and output zero-buffers stay resident on device.
"""

import numpy as np

HEADS = 8
E = 16
EPS = 1e-6
L = 512
NSEQ = 64
F = 128
CH = HEADS * E  # 128 output channels
C = 128         # time chunk
NC = L // C
W17 = 17 * HEADS  # 136: per-head [num(16) | den(1)] interleaved width
AMAX_FLOOR = 1e-4

S = 4                  # node-axis slices pipelined per kernel() call
NS = NSEQ // S         # nodes per core per slice-call
NCOL = NS * NC         # scale columns per slice: one per (n, c) tile

# int8 blob row layout (rows x 128 bytes), input side
R_X = L * NS                         # xq int8 rows, row = t*NS + n
R_SC = C * NCOL * 2 // F             # f16 scales region rows
R_W = (F * CH * 2) // F              # one f16 weight matrix region rows
R_B = (3 * CH * 2 + F - 1) // F      # three f16 biases region rows
R_IN = R_X + R_SC + 3 * R_W + R_B
R_OUT = R_X + R_SC                   # y8 rows + f16 out-scales rows


def build_nc(debug=False):
    """Build the per-core Bass module for one slice-call (NS node seqs)."""
    from contextlib import ExitStack

    import concourse.bacc as bacc
    import concourse.mybir as mybir
    import concourse.tile as tile

    i8 = mybir.dt.int8
    f16 = mybir.dt.float16
    f32 = mybir.dt.float32
    Relu = mybir.ActivationFunctionType.Relu
    Exp = mybir.ActivationFunctionType.Exp
    AluOp = mybir.AluOpType
    AX = mybir.AxisListType.X

    nc = bacc.Bacc(
        "TRN2",
        target_bir_lowering=False,
        debug=debug,
        enable_asserts=False,
        num_devices=8,
    )

    blob = nc.dram_tensor("blob", (R_IN, F), i8, kind="ExternalInput").ap()
    cmask = nc.dram_tensor("cmask", (C, C), f16, kind="ExternalInput").ap()
    bdmask = nc.dram_tensor("bdmask", (CH, W17), f32, kind="ExternalInput").ap()
    hmask = nc.dram_tensor("hmask", (CH, HEADS), f32, kind="ExternalInput").ap()
    yout = nc.dram_tensor("yout", (R_OUT, F), i8, kind="ExternalOutput").ap()

    o = R_X
    xsc = blob[o:o + R_SC, :].bitcast(f16).rearrange("(a b) c -> a (b c)", b=2)
    o += R_SC
    wq = blob[o:o + R_W, :].bitcast(f16).rearrange("(a b) c -> a (b c)", b=2)
    o += R_W
    wk = blob[o:o + R_W, :].bitcast(f16).rearrange("(a b) c -> a (b c)", b=2)
    o += R_W
    wv = blob[o:o + R_W, :].bitcast(f16).rearrange("(a b) c -> a (b c)", b=2)
    o += R_W
    bqkv = blob[o:o + R_B, :].bitcast(f16).rearrange("(a b) c -> a (b c)", b=2)

    x3 = blob[0:R_X, :].rearrange("(t n) f -> t n f", n=NS)
    y3 = yout[0:R_X, :].rearrange("(t n) f -> t n f", n=NS)
    osc_out = yout[R_X:R_OUT, :].bitcast(f16).rearrange("(a b) c -> a (b c)", b=2)

    with tile.TileContext(nc) as tc, ExitStack() as ctx:
        cpool = ctx.enter_context(tc.tile_pool(name="consts", bufs=1))
        wq_sb = cpool.tile([F, CH], f16, tag="wq")
        wk_sb = cpool.tile([F, CH], f16, tag="wk")
        wv_sb = cpool.tile([F, CH], f16, tag="wv")
        nc.scalar.dma_start(wq_sb[:], wq)
        nc.scalar.dma_start(wk_sb[:], wk)
        nc.scalar.dma_start(wv_sb[:], wv)
        bq_sb = cpool.tile([1, CH], f16, tag="bq")
        bk_sb = cpool.tile([1, CH], f16, tag="bk")
        bv_sb = cpool.tile([1, CH], f16, tag="bv")
        nc.scalar.dma_start(bq_sb[:], bqkv[0:1, :])
        nc.scalar.dma_start(bk_sb[:], bqkv[1:2, :])
        nc.scalar.dma_start(bv_sb[:], bqkv[2:3, :])
        ones_sb = cpool.tile([1, C], f16, tag="ones")
        nc.vector.memset(ones_sb[:], 1.0)
        cm_sb = cpool.tile([C, C], f16, tag="cmask")
        nc.scalar.dma_start(cm_sb[:], cmask)
        bd_sb = cpool.tile([CH, W17], f32, tag="bdmask")
        nc.scalar.dma_start(bd_sb[:], bdmask)
        hm_sb = cpool.tile([CH, HEADS], f32, tag="hmask")
        nc.scalar.dma_start(hm_sb[:], hmask)
        xsc16 = cpool.tile([C, NCOL], f16, tag="xsc16")
        nc.scalar.dma_start(xsc16[:], xsc)
        xsc_sb = cpool.tile([C, NCOL], f32, tag="xsc")
        nc.vector.tensor_copy(xsc_sb[:], xsc16[:])
        osc_sb = cpool.tile([C, NCOL], f16, tag="osc")

        xpool = ctx.enter_context(tc.tile_pool(name="x", bufs=3))
        phipool = ctx.enter_context(tc.tile_pool(name="phi", bufs=3))
        spool = ctx.enter_context(tc.tile_pool(name="sacc", bufs=1))
        tpool = ctx.enter_context(tc.tile_pool(name="tmp", bufs=2))
        opool = ctx.enter_context(tc.tile_pool(name="out", bufs=3))
        ps_proj = ctx.enter_context(tc.tile_pool(name="psproj", bufs=4, space="PSUM"))
        ps_at = ctx.enter_context(tc.tile_pool(name="psat", bufs=1, space="PSUM"))
        ps_acc = ctx.enter_context(tc.tile_pool(name="psacc", bufs=1, space="PSUM"))
        ps_inta = ctx.enter_context(tc.tile_pool(name="psinta", bufs=1, space="PSUM"))
        ps_g = ctx.enter_context(tc.tile_pool(name="psg", bufs=1, space="PSUM"))

        def phi(dst, ps):
            # phi(x) = elu(x) + 1 = relu(x) + exp(min(x, 0))
            shape = [ps.shape[0], ps.shape[1]]
            a = tpool.tile(shape, f32, tag="phia")
            b = tpool.tile(shape, f32, tag="phib")
            nc.scalar.activation(a[:], ps[:], Relu)
            nc.vector.tensor_scalar_min(b[:], ps[:], 0.0)
            nc.scalar.activation(b[:], b[:], Exp)
            nc.vector.tensor_add(dst[:], a[:], b[:])

        for n in range(NS):
            S_acc = spool.tile([CH, W17], f32, tag="sacc")
            nc.vector.memset(S_acc[:], 0.0)
            for c in range(NC):
                col = n * NC + c
                # load int8 chunk [tok, F], dequant per-token, transpose to [F, tok]
                xi8 = xpool.tile([C, F], i8, tag="xi8")
                nc.scalar.dma_start(xi8[:], x3[c * C:(c + 1) * C, n, :])
                x16 = xpool.tile([C, F], f16, tag="x16")
                nc.vector.tensor_scalar_mul(x16[:], xi8[:], xsc_sb[:, col:col + 1])
                xT = xpool.tile([F, C], f16, tag="xT")
                nc.sync.dma_start(xT[:], x16[:], transpose=True)

                # projections (+ rank-1 bias add)
                qT_ps = ps_proj.tile([CH, C], f32, tag="proj")
                kT_ps = ps_proj.tile([CH, C], f32, tag="proj")
                kt_ps = ps_proj.tile([C, CH], f32, tag="proj")
                vt_ps = ps_proj.tile([C, CH], f32, tag="proj")
                nc.tensor.matmul(qT_ps[:], wq_sb[:], xT[:], start=True, stop=False)
                nc.tensor.matmul(qT_ps[:], bq_sb[:], ones_sb[:], start=False, stop=True)
                nc.tensor.matmul(kT_ps[:], wk_sb[:], xT[:], start=True, stop=False)
                nc.tensor.matmul(kT_ps[:], bk_sb[:], ones_sb[:], start=False, stop=True)
                nc.tensor.matmul(kt_ps[:], xT[:], wk_sb[:], start=True, stop=False)
                nc.tensor.matmul(kt_ps[:], ones_sb[:], bk_sb[:], start=False, stop=True)
                nc.tensor.matmul(vt_ps[:], xT[:], wv_sb[:], start=True, stop=False)
                nc.tensor.matmul(vt_ps[:], ones_sb[:], bv_sb[:], start=False, stop=True)

                q16 = phipool.tile([CH, C], f16, tag="q16")   # phi(q)^T [chan, tok]
                k16 = phipool.tile([CH, C], f16, tag="k16")   # phi(k)^T [chan, tok]
                kt16 = phipool.tile([C, CH], f16, tag="kt16")  # phi(k) [tok, chan]
                phi(q16, qT_ps)
                phi(k16, kT_ps)
                phi(kt16, kt_ps)

                # v_aug [tok, 136]: per head h cols h*17..h*17+15 = v_h, col h*17+16 = 1
                vaug = phipool.tile([C, W17], f16, tag="vaug")
                va = vaug[:].rearrange("p (h j) -> p h j", j=17)
                vs = vt_ps[:].rearrange("p (h j) -> p h j", j=16)
                nc.vector.tensor_copy(va[:, :, 0:16], vs)
                nc.vector.memset(va[:, :, 16:17], 1.0)

                # inter-chunk: acc[t, :] = phi(q)_t @ S_prev (block-diag interleaved)
                s16 = phipool.tile([CH, W17], f16, tag="s16")
                nc.vector.tensor_copy(s16[:], S_acc[:])
                acc_ps = ps_acc.tile([C, W17], f32, tag="acc")
                nc.tensor.matmul(acc_ps[:], q16[:], s16[:], start=True, stop=True)

                # intra-chunk per head: A^T = (k.head_mask)^T q (K=128, head-
                # masked k zeroes cross-head terms), mask causal, A_m^T.T@[v|1]
                inta_ps = ps_inta.tile([C, W17], f32, tag="inta")
                for h in range(HEADS):
                    kh = tpool.tile([CH, C], f16, tag="kh")
                    nc.vector.tensor_scalar_mul(kh[:], k16[:], hm_sb[:, h:h + 1])
                    at_ps = ps_at.tile([C, C], f32, tag="at")
                    nc.tensor.matmul(
                        at_ps[:], kh[:], q16[:], start=True, stop=True,
                    )
                    am = tpool.tile([C, C], f16, tag="am")
                    nc.vector.tensor_mul(am[:], at_ps[:], cm_sb[:])
                    nc.tensor.matmul(
                        inta_ps[:, h * 17:h * 17 + 17],
                        am[:],
                        vaug[:, h * 17:h * 17 + 17],
                        start=True, stop=True,
                    )

                # KV gram for this chunk + masked accumulate into S
                g_ps = ps_g.tile([CH, W17], f32, tag="g")
                nc.tensor.matmul(g_ps[:], kt16[:], vaug[:], start=True, stop=True)
                gt = tpool.tile([CH, W17], f32, tag="gt")
                nc.vector.tensor_mul(gt[:], g_ps[:], bd_sb[:])
                nc.vector.tensor_add(S_acc[:], S_acc[:], gt[:])

                # normalize: out = (num_inter + num_intra) / (den_i + den_x + eps)
                # DVE reads at most one PSUM operand: stage intra to SBUF first.
                inta_sb = tpool.tile([C, W17], f32, tag="intasb")
                nc.vector.tensor_copy(inta_sb[:], inta_ps[:])
                accv = acc_ps[:].rearrange("p (h j) -> p h j", j=17)
                intav = inta_sb[:].rearrange("p (h j) -> p h j", j=17)
                den = tpool.tile([C, HEADS], f32, tag="den")
                dv = den[:].rearrange("p (h j) -> p h j", j=1)
                nc.vector.scalar_tensor_tensor(
                    dv, accv[:, :, 16:17], EPS, intav[:, :, 16:17],
                    op0=AluOp.add, op1=AluOp.add,
                )
                rec = tpool.tile([C, HEADS], f32, tag="rec")
                nc.vector.reciprocal(rec[:], den[:])
                out_f = opool.tile([C, CH], f32, tag="outf")
                for h in range(HEADS):
                    nsum = tpool.tile([C, E], f32, tag="nsum")
                    nc.vector.tensor_add(
                        nsum[:],
                        acc_ps[:, h * 17:h * 17 + 16],
                        inta_sb[:, h * 17:h * 17 + 16],
                    )
                    nc.vector.tensor_scalar_mul(
                        out_f[:, h * 16:(h + 1) * 16],
                        nsum[:],
                        rec[:, h:h + 1],
                    )

                # int8 quantize per token: amax, scale out, store scale
                amax = tpool.tile([C, 1], f32, tag="amax")
                nc.vector.reduce_max(
                    amax[:], out_f[:], axis=AX, apply_absolute_value=True
                )
                nc.vector.tensor_scalar_max(amax[:], amax[:], AMAX_FLOOR)
                nc.vector.tensor_scalar_mul(
                    osc_sb[:, col:col + 1], amax[:], 1.0 / 127.0
                )
                r8 = tpool.tile([C, 1], f32, tag="r8")
                nc.vector.reciprocal(r8[:], amax[:])
                y8t = opool.tile([C, CH], i8, tag="y8t")
                nc.vector.tensor_scalar(
                    y8t[:], out_f[:], r8[:, 0:1], 127.0,
                    op0=AluOp.mult, op1=AluOp.mult,
                )
                nc.gpsimd.dma_start(y3[c * C:(c + 1) * C, n, :], y8t[:])

        nc.gpsimd.dma_start(osc_out, osc_sb[:])

    nc.compile()
    return nc


def _consts():
    cmask = np.triu(np.ones((C, C), np.float16))  # cmask[s,t] = 1 if s<=t
    bd = np.zeros((CH, W17), np.float32)
    for h in range(HEADS):
        bd[h * 16:(h + 1) * 16, h * 17:(h + 1) * 17] = 1.0
    hm = np.zeros((CH, HEADS), np.float32)
    for h in range(HEADS):
        hm[h * 16:(h + 1) * 16, h] = 1.0
    return cmask, bd, hm


_RUNNER = None


def _make_runner():
    import jax
    from jax.sharding import Mesh, NamedSharding, PartitionSpec

    try:
        from jax.experimental.shard_map import shard_map
    except ImportError:
        from jax.shard_map import shard_map

    import concourse.mybir as mybir
    from concourse.bass2jax import (
        _bass_exec_p,
        install_neuronx_cc_hook,
        partition_id_tensor,
    )

    install_neuronx_cc_hook()
    nc = build_nc()

    partition_name = (
        nc.partition_id_tensor.name if nc.partition_id_tensor is not None else None
    )
    in_names: list[str] = []
    out_names: list[str] = []
    out_avals = []
    zero_outs = []
    for alloc in nc.m.functions[0].allocations:
        if not isinstance(alloc, mybir.MemoryLocationSet):
            continue
        name = alloc.memorylocations[0].name
        if alloc.kind == "ExternalInput":
            if name != partition_name:
                in_names.append(name)
        elif alloc.kind == "ExternalOutput":
            out_names.append(name)
            shape = tuple(alloc.tensor_shape)
            dtype = mybir.dt.np(alloc.dtype)
            out_avals.append(jax.core.ShapedArray(shape, dtype))
            zero_outs.append(np.zeros((8 * shape[0], *shape[1:]), dtype))
    n_params = len(in_names)
    all_in_names = in_names + out_names
    if partition_name is not None:
        all_in_names = all_in_names + [partition_name]

    def _body(*args):
        operands = list(args)
        if partition_name is not None:
            operands.append(partition_id_tensor())
        outs = _bass_exec_p.bind(
            *operands,
            out_avals=tuple(out_avals),
            in_names=tuple(all_in_names),
            out_names=tuple(out_names),
            lowering_input_output_aliases=(),
            sim_require_finite=True,
            sim_require_nnan=True,
            nc=nc,
        )
        return tuple(outs)

    devices = jax.devices()[:8]
    mesh = Mesh(np.asarray(devices), ("core",))
    spec = NamedSharding(mesh, PartitionSpec("core"))
    nin = n_params + len(out_names)
    sharded = jax.jit(
        shard_map(
            _body,
            mesh=mesh,
            in_specs=(PartitionSpec("core"),) * nin,
            out_specs=(PartitionSpec("core"),) * len(out_names),
            check_rep=False,
        ),
        keep_unused=True,
    )

    cmask, bd, hm = _consts()
    persist = {
        "cmask": jax.device_put(np.tile(cmask, (8, 1)), spec),
        "bdmask": jax.device_put(np.tile(bd, (8, 1)), spec),
        "hmask": jax.device_put(np.tile(hm, (8, 1)), spec),
    }
    zeros_dev = [jax.device_put(z, spec) for z in zero_outs]
    oi = {nm: i for i, nm in enumerate(out_names)}
    yi = oi["yout"]

    # persistent per-slice input blobs (numpy, reused across calls)
    blobs = [np.empty((8 * R_IN, F), np.int8) for _ in range(S)]

    O_SC = R_X
    O_WQ = O_SC + R_SC
    O_WK = O_WQ + R_W
    O_WV = O_WK + R_W
    O_B = O_WV + R_W

    def pack_slice(s, query, wbytes):
        """Quantize node-slice s of all 8 batches into blobs[s]."""
        blob = blobs[s]
        n0 = s * NS
        for b in range(8):
            base = b * R_IN
            xs = query[b, :, n0:n0 + NS, :]  # (L, NS, F) f32 view
            amax = np.maximum(xs.max(axis=2), -xs.min(axis=2))  # (L, NS)
            np.maximum(amax, 1e-12, out=amax)
            inv = np.float32(127.0) / amax
            q = xs * inv[:, :, None]
            np.rint(q, out=q)
            xv = blob[base:base + R_X, :].reshape(L, NS, F)
            np.copyto(xv, q, casting="unsafe")
            # scales f16, layout [j, n*NC+c] from amax[t= c*C+j, n]
            sc = amax * np.float32(1.0 / 127.0)
            sct = sc.reshape(NC, C, NS).transpose(1, 2, 0).reshape(C, NCOL)
            scv = blob[base + O_SC:base + O_WQ, :].view(np.float16)
            scv.reshape(C, NCOL)[:] = sct
            blob[base + O_WQ:base + O_B + R_B, :] = wbytes
        return blob

    def unpack_slice(s, y, res):
        """Dequantize slice-call output y into res[:, :, n0:n1, :]."""
        n0 = s * NS
        for b in range(8):
            base = b * R_OUT
            yb = y[base:base + R_X, :].reshape(L, NS, CH)
            oscb = y[base + R_X:base + R_OUT, :].view(np.float16)
            sf = (
                oscb.reshape(C, NS, NC)
                .transpose(2, 0, 1)
                .reshape(L, NS)
                .astype(np.float32)
            )
            np.multiply(yb, sf[:, :, None], out=res[b, :, n0:n0 + NS, :])

    def run(query, Wq, bq_, Wk, bk_, Wv, bv_):
        query = np.ascontiguousarray(np.asarray(query, np.float32))
        # weight+bias byte block, shared by every core/slice
        wbytes = np.empty((3 * R_W + R_B, F), np.int8)
        wv16 = wbytes[0:3 * R_W, :].view(np.float16).reshape(3, F, CH)
        wv16[0] = Wq
        wv16[1] = Wk
        wv16[2] = Wv
        bv16 = wbytes[3 * R_W:, :].view(np.float16).reshape(3, CH)
        bv16[0] = bq_
        bv16[1] = bk_
        bv16[2] = bv_

        outs = []
        for s in range(S):
            blob = pack_slice(s, query, wbytes)
            out = sharded(blob, *(persist[nm] for nm in in_names[1:]), *zeros_dev)
            y = out[yi]
            y.copy_to_host_async()
            outs.append(y)

        res = np.empty((8, L, NSEQ, CH), np.float32)
        for s in range(S):
            y = np.asarray(outs[s])
            unpack_slice(s, y, res)
        return res

    # in_names sanity: blob must be first, rest must be persisted consts
    assert in_names[0] == "blob", in_names
    assert all(nm in persist for nm in in_names[1:]), in_names

    return run


def kernel(query, Wq, bq, Wk, bk, Wv, bv):
    global _RUNNER
    if _RUNNER is None:
        _RUNNER = _make_runner()
    return _RUNNER(np.asarray(query, np.float32), Wq, bq, Wk, bk, Wv, bv)
